# revision 19
# baseline (speedup 1.0000x reference)
"""Sharded kNN (ArgDistanceMeasure) on 8 TRN2 NeuronCores.

Strategy (FAISS-style sharded kNN):
  - b (the database, [65536, 512]) is sharded row-wise across 8 cores
    (8192 rows each); a (queries, [2048, 512]) is replicated.
  - Ranking identity: argmin_j ||a_i - b_j + eps||^2 over j only needs the
    column-dependent part  score[i,j] = 2*a_i.b_j - (||b_j||^2 - 2*eps*sum(b_j)),
    maximized.  The row-constant terms (||a_i||^2 etc.) don't affect per-row
    ranking.
  - Each core computes score = (2a)^T-GEMM minus a per-column bias folded into
    the PSUM accumulation as a K=1 rank-1 matmul (ones ⊗ -c), using float32r
    (full-rate fp32 path on the PE array).
  - VectorE max/max_index extract the top-8 candidates per 2048-column chunk
    (32 candidates/core, 256 global) directly from PSUM.
  - Host gathers the 8 small outputs, recomputes the exact fp32 reference
    distance for the 256 candidates/query, picks the final top-n with the
    reference's tie-break, and applies the reference's index bookkeeping.
"""

import numpy as np

NA, D, NB = 2048, 512, 65536
NCORES = 8
NB_SHARD = NB // NCORES  # 8192
CHUNK = 2048             # DVE scan segment width (4 PSUM banks)
TOP = 8                  # top-8 per chunk (vector.max width)
EPS = 1e-6


def build_kernel(na=NA, nb_shard=NB_SHARD, chunk=CHUNK):
    import concourse.mybir as mybir
    from concourse import bacc
    from concourse.tile import TileContext

    FR = mybir.dt.float32r
    F32 = mybir.dt.float32
    U32 = mybir.dt.uint32

    nseg = nb_shard // chunk
    nsub = chunk // 512
    kt = D // 128
    mt = na // 128

    # Bacc (not plain Bass): its compile() pipeline moves matmul waits onto
    # ldweights and splits multi-wait sync via event semaphores — TRN2
    # instructions encode at most ONE sync wait.
    nc = bacc.Bacc()
    # Matmul APs must sit at base partition 0/32/64, so the per-512-segment
    # bias vectors are packed round-robin onto those three partitions with a
    # matching all-ones lhsT row on each.
    nsegs512 = nb_shard // 512
    cneg_cols = ((nsegs512 + 2) // 3) * 512
    # All inputs are host-packed into ONE [128, X] blob loaded by a single
    # dma_start: one DMA fans out across all 16 SDMA engines (~425 GB/s) and
    # leaves downstream instructions exactly one DMA-queue wait (the
    # per-instruction sync-wait slots on TRN2 are tiny).
    bt_cols = kt * nb_shard
    at_cols = kt * na
    blob_cols = bt_cols + at_cols + cneg_cols + 128
    blob = nc.declare_dram_parameter("blob", [128, blob_cols], FR, isOutput=False)
    out_val = nc.declare_dram_parameter("out_val", [na, nseg * TOP], F32, isOutput=True)
    out_idx = nc.declare_dram_parameter("out_idx", [na, nseg * TOP], U32, isOutput=True)

    with TileContext(nc) as tc:
        with (
            tc.tile_pool(name="weights", bufs=1) as wpool,
            tc.tile_pool(name="psum", bufs=2, space="PSUM") as ppool,
            tc.tile_pool(name="win", bufs=4) as winpool,
        ):
            sb = wpool.tile([128, blob_cols], FR, tag="blob")
            nc.sync.dma_start(out=sb, in_=blob[:, :])
            bts = [sb[:, k * nb_shard : (k + 1) * nb_shard] for k in range(kt)]
            ats = [
                sb[:, bt_cols + k * na : bt_cols + (k + 1) * na] for k in range(kt)
            ]
            cn = sb[:, bt_cols + at_cols : bt_cols + at_cols + cneg_cols]
            ones = sb[:, bt_cols + at_cols + cneg_cols : blob_cols]

            for m in range(mt):
                wv = winpool.tile([128, nseg * TOP], F32, tag="wval")
                wi = winpool.tile([128, nseg * TOP], U32, tag="widx")
                for s in range(nseg):
                    ps = ppool.tile([128, chunk], F32, tag="score")
                    for k in range(kt):
                        for j in range(nsub):
                            nc.tensor.matmul(
                                ps[:, j * 512 : (j + 1) * 512],
                                ats[k][:, m * 128 : (m + 1) * 128],
                                bts[k][:, s * chunk + j * 512 : s * chunk + (j + 1) * 512],
                                start=(k == 0),
                                stop=False,
                            )
                    for j in range(nsub):
                        seg = s * nsub + j
                        bp = 32 * (seg % 3)
                        off = (seg // 3) * 512
                        nc.tensor.matmul(
                            ps[:, j * 512 : (j + 1) * 512],
                            ones[bp : bp + 1, :],
                            cn[bp : bp + 1, off : off + 512],
                            start=False,
                            stop=True,
                        )
                    nc.vector.max(out=wv[:, s * TOP : (s + 1) * TOP], in_=ps)
                    nc.vector.max_index(
                        out=wi[:, s * TOP : (s + 1) * TOP],
                        in_max=wv[:, s * TOP : (s + 1) * TOP],
                        in_values=ps,
                    )
                nc.sync.dma_start(out=out_val[m * 128 : (m + 1) * 128, :], in_=wv)
                nc.sync.dma_start(out=out_idx[m * 128 : (m + 1) * 128, :], in_=wi)
    nc.compile()
    return nc


def pack_cneg(c_shard):
    """Pack -c per 512-segment round-robin onto partitions 0/32/64."""
    nsegs512 = c_shard.shape[0] // 512
    cols = ((nsegs512 + 2) // 3) * 512
    arr = np.zeros((65, cols), np.float32)
    for s in range(nsegs512):
        bp = 32 * (s % 3)
        off = (s // 3) * 512
        arr[bp, off : off + 512] = -c_shard[s * 512 : (s + 1) * 512]
    return arr


def make_in_maps(a, b):
    kt = D // 128
    aT2 = (2.0 * a).T.astype(np.float32)              # [512, NA]
    aTp = np.concatenate(
        [aT2[k * 128 : (k + 1) * 128, :] for k in range(kt)], axis=1
    )                                                 # [128, kt*NA]
    bT_full = b.T.astype(np.float32)                  # [512, NB]
    b2 = np.einsum("ij,ij->i", b, b)
    sb = b.sum(axis=1)
    c = (b2 - np.float32(2.0 * EPS) * sb).astype(np.float32)
    ones = np.zeros((128, 128), np.float32)
    ones[[0, 32, 64], :] = 1.0
    in_maps = []
    for core in range(NCORES):
        sl = slice(core * NB_SHARD, (core + 1) * NB_SHARD)
        bT = bT_full[:, sl]
        bTp = np.concatenate(
            [bT[k * 128 : (k + 1) * 128, :] for k in range(kt)], axis=1
        )                                             # [128, kt*NB_SHARD]
        cneg128 = np.zeros((128, pack_cneg(c[sl]).shape[1]), np.float32)
        cneg128[:65] = pack_cneg(c[sl])
        blob = np.ascontiguousarray(
            np.concatenate([bTp, aTp, cneg128, ones], axis=1)
        )
        in_maps.append({"blob": blob})
    return in_maps


def merge_results(a, b, n, b_batch_size, results):
    """Gather per-core candidates, refine with the exact fp32 reference
    distance, pick final top-n (ties -> lowest index), apply the reference's
    buggy index bookkeeping."""
    nseg = NB_SHARD // CHUNK
    cand = []
    for core in range(NCORES):
        gi = results[core]["out_idx"].astype(np.int64)
        for s in range(nseg):
            gi[:, s * TOP : (s + 1) * TOP] += core * NB_SHARD + s * CHUNK
        cand.append(gi)
    cand = np.concatenate(cand, axis=1)  # [NA, 256]

    a2 = np.sum(a * a, axis=1)
    sa = np.sum(a, axis=1)
    b2 = np.sum(b * b, axis=1)
    sb = np.sum(b, axis=1)
    na, d = a.shape
    out = np.empty((na, n), dtype=np.int64)
    CHQ = 256
    eps = np.float32(EPS)
    for q0 in range(0, na, CHQ):
        q1 = min(q0 + CHQ, na)
        Cc = cand[q0:q1]
        Bc = b[Cc]  # [q, 256, 512]
        cross = np.einsum("qd,qkd->qk", a[q0:q1], Bc).astype(np.float32)
        sq = (
            a2[q0:q1, None]
            + b2[Cc]
            - np.float32(2.0) * cross
            + np.float32(2.0) * eps * (sa[q0:q1, None] - sb[Cc])
            + np.float32(d) * eps * eps
        )
        dist = np.sqrt(np.maximum(sq, np.float32(0.0)))
        ordr = np.lexsort((Cc, dist), axis=1)[:, :n]
        rows = np.arange(q1 - q0)[:, None]
        out[q0:q1] = Cc[rows, ordr]
    buggy = (out % b_batch_size) + (out // b_batch_size)
    return buggy.astype(np.int32)


def kernel(a, b, n, b_batch_size, trace=False):
    from concourse.bass_utils import run_bass_kernel_spmd

    a = np.ascontiguousarray(np.asarray(a, dtype=np.float32))
    b = np.ascontiguousarray(np.asarray(b, dtype=np.float32))
    n = int(n)
    b_batch_size = int(b_batch_size)

    nc = build_kernel()
    in_maps = make_in_maps(a, b)
    res = run_bass_kernel_spmd(
        nc, in_maps, core_ids=list(range(NCORES)), trace=trace
    )
    out = merge_results(a, b, n, b_batch_size, res.results)
    if trace:
        return out, res
    return out
